# revision 1
# baseline (speedup 1.0000x reference)
"""MHA kernel for Trainium2, 8-core tensor-parallel (2 heads per core).

Problem (hardcoded): x [2, 2048, 1024] fp32, Wq/Wk/Wv/Wo [1024, 1024],
bq/bk/bv/bo [1024], H=16 heads, DH=64.  out = MHA(x).

Sharding: heads are split 8 ways (2 heads = 128 proj columns per core).
Each core computes its heads' attention output and a partial output
projection (row-parallel Wo); the host sums the 8 partials and adds the
closed-form bias terms (bv @ Wo + bo).

Per-core device pipeline (all big matmuls bf16 in / fp32 accumulate):
  1. Q^T, K^T [128, 4096] = W.T @ x.T            (contract D, psum N=512)
  2. V token-major [tok, 64] per (b, h, ktile), augmented with a ones
     column -> lhsT [128, 65] slots
  3. S^T tiles [128 k, 512 q] = K Q^T; the two heads' K=64 matmuls sit
     on row-groups 0-1 / 2-3 so the PE packs them concurrently
  4. P^T = exp(S^T / 8) on ScalarE (scores in [-3.6, 3.6], no max pass)
  5. O_raw^T + denom = [V|1].T @ P^T             (psum [65, 512])
  6. r2 = 1/denom (both heads), broadcast via one K=2 fp32 matmul
     (eye2), O_norm^T = O_raw^T * r
  7. out^partial [tok 128, 512] = O_norm^T.T @ Wo  (token-major, fp32)
"""

import numpy as np
import ml_dtypes

D = 1024
T = 4096          # B*S tokens
S = 2048
B = 2
NH = 2            # heads per core
DH = 64
NCORES = 8
SCALE = 0.125     # 1/sqrt(DH)

_CACHE = {}


def _build_nc(reps=1):
    import concourse.bacc as bacc
    import concourse.mybir as mybir
    import concourse.tile as tile
    from concourse.hw_specs import get_activation_tables as _gat

    # Pin Exp and Ln to the one table set that holds both, so the
    # table-load placement pass emits a single ACT_TABLE_LOAD instead of
    # thrashing between exp_and_others and natural_log every combo.
    def _pinned_tables(arch):
        out = {}
        for k, fns in _gat(arch).items():
            if k != "natural_log_exp_and_others":
                fns = {f for f in fns if f.name not in ("Exp", "Ln")}
            out[k] = fns
        return out
    bacc.get_activation_tables = _pinned_tables

    dt = mybir.dt
    f32, bf16 = dt.float32, dt.bfloat16

    nc = bacc.Bacc("TRN2", target_bir_lowering=False, debug=False,
                   num_devices=NCORES)

    xT = nc.dram_tensor("xT", [D, T], bf16, kind="ExternalInput")
    wq_d = nc.dram_tensor("wq", [D, 128], bf16, kind="ExternalInput")
    wk_d = nc.dram_tensor("wk", [D, 128], bf16, kind="ExternalInput")
    wv_d = nc.dram_tensor("wv", [D, 128], bf16, kind="ExternalInput")
    wo_d = nc.dram_tensor("wo", [128, D], bf16, kind="ExternalInput")
    bq_d = nc.dram_tensor("bq", [128, 1], f32, kind="ExternalInput")
    bk_d = nc.dram_tensor("bk", [128, 1], f32, kind="ExternalInput")
    outp = nc.dram_tensor("outp", [T, D], f32, kind="ExternalOutput")

    NKT = S // 128        # 16 key tiles per batch
    NQC = S // 512        # 4 query chunks per batch
    NCK = T // 512        # 8 x^T column chunks
    VSLOT = DH + 1        # 65: V columns + ones column

    with tile.TileContext(nc) as tc:
      for _rep in range(reps):
        with (
            tc.tile_pool(name="persist", bufs=1) as pp,
            tc.tile_pool(name="pt", bufs=2) as ptp,
            tc.tile_pool(name="onorm", bufs=2) as onp,
            tc.tile_pool(name="oraw", bufs=2) as orp,
            tc.tile_pool(name="recip", bufs=2) as rcp,
            tc.tile_pool(name="outsb", bufs=3) as osp,
        ):
            # ---- constants / weights ----
            wq = pp.tile([128, D], bf16, tag="wq")
            wk = pp.tile([128, D], bf16, tag="wk")
            wv = pp.tile([128, D], bf16, tag="wv")
            wo = pp.tile([128, D], bf16, tag="wo")
            for w_sb, w_dr in ((wq, wq_d), (wk, wk_d), (wv, wv_d)):
                nc.sync.dma_start(
                    out=w_sb.rearrange("p (t c) -> p t c", c=128),
                    in_=w_dr.ap().rearrange("(t p) c -> p t c", p=128),
                )
            nc.sync.dma_start(out=wo[:, :], in_=wo_d.ap()[:, :])
            bq = pp.tile([128, 1], f32, tag="bq")
            bk = pp.tile([128, 1], f32, tag="bk")
            nc.sync.dma_start(out=bq[:, :], in_=bq_d.ap()[:, :])
            nc.sync.dma_start(out=bk[:, :], in_=bk_d.ap()[:, :])

            with tc.tile_pool(name="mm_ps", bufs=2, space="PSUM") as mmp:
              # ---- x^T d-tiles, loaded in 512-col chunks so the QKV
              # matmuls can start as soon as chunk 0 of all 8 d-tiles lands
              xt = [pp.tile([128, T], bf16, tag=f"xt{d}", name=f"xt{d}")
                    for d in range(8)]
              for nck in range(NCK):
                  cs = slice(nck * 512, (nck + 1) * 512)
                  for d in range(8):
                      nc.sync.dma_start(
                          out=xt[d][:, cs],
                          in_=xT.ap()[d * 128:(d + 1) * 128, cs])

              # ---- Q^T / K^T projections ----
              qt = pp.tile([128, T], bf16, tag="qt")
              kt = pp.tile([128, T], bf16, tag="kt")
              for nck in range(NCK):
                  cs = slice(nck * 512, (nck + 1) * 512)
                  for proj_sb, w_sb, b_sb in ((qt, wq, bq), (kt, wk, bk)):
                      w3 = w_sb.rearrange("p (t c) -> p t c", c=128)
                      ps = mmp.tile([128, 512], f32, tag="mm")
                      for d in range(8):
                          nc.tensor.matmul(
                              ps[:, :], w3[:, d, :], xt[d][:, cs],
                              start=(d == 0), stop=(d == 7),
                          )
                      nc.vector.tensor_scalar_add(proj_sb[:, cs], ps[:, :],
                                                  b_sb[:, :])

              # ---- V token-major (augmented with ones col) ----
              # per batch: [128 tok, (h, kt) slots of 65]
              vtm = []
              for b in range(B):
                  v_sb = pp.tile([128, NH * NKT * VSLOT], bf16, tag=f"v{b}")
                  v4 = v_sb.rearrange("p (h k c) -> p h k c", h=NH, k=NKT)
                  nc.vector.memset(v4[:, :, :, DH:DH + 1], 1.0)
                  vtm.append(v_sb)
              wv3 = wv.rearrange("p (t c) -> p t c", c=128)
              for b in range(B):
                  v4 = vtm[b].rearrange("p (h k c) -> p h k c", h=NH, k=NKT)
                  for k in range(NKT):
                      tok0 = b * S + k * 128
                      ps = mmp.tile([128, 128], f32, tag="mm")
                      for d in range(8):
                          nc.tensor.matmul(
                              ps[:, :], xt[d][:, tok0:tok0 + 128], wv3[:, d, :],
                              start=(d == 0), stop=(d == 7),
                          )
                      nc.vector.tensor_copy(
                          v4[:, :, k, 0:DH],
                          ps.rearrange("p (h c) -> p h c", h=NH)[:, :, :],
                      )

            with (
                tc.tile_pool(name="st_ps", bufs=3, space="PSUM") as stp,
                tc.tile_pool(name="av_ps", bufs=2, space="PSUM") as avp,
            ):
              # ---- attention per (b, qc), heads interleaved ----
              # The outproj matmuls of combo i are emitted AFTER combo i+1's
              # scores/AV matmuls: PE is in-order, and this gives the
              # reciprocal/broadcast chain a full combo to finish without
              # stalling the PE (stalls > 3.4us re-throttle the PE clock).
              def emit_outproj(q0, onorm):
                  for s4 in range(4):
                      for jc in range(2):
                          op = avp.tile([128, 512], f32, tag="av",
                                        name=f"op{q0}_{s4}_{jc}")
                          nc.tensor.matmul(
                              op[:, :], onorm[:, s4 * 128:(s4 + 1) * 128],
                              wo[:, jc * 512:(jc + 1) * 512],
                              start=True, stop=True,
                          )
                          osb = osp.tile([128, 512], f32, tag="outsb",
                                         name=f"osb{q0}_{s4}_{jc}")
                          nc.vector.tensor_copy(osb[:, :], op[:, :])
                          r0 = q0 + s4 * 128
                          nc.sync.dma_start(
                              out=outp.ap()[r0:r0 + 128,
                                            jc * 512:(jc + 1) * 512],
                              in_=osb[:, :],
                          )

              pending = None
              for b in range(B):
                  v4 = vtm[b].rearrange("p (h k c) -> p h k c", h=NH, k=NKT)
                  for qc in range(NQC):
                      q0 = b * S + qc * 512
                      onorm = onp.tile([128, 512], bf16, tag="onorm",
                                       name=f"onorm{b}_{qc}")
                      pt = ptp.tile([128, NH * NKT * 512], bf16, tag="pt",
                                    name=f"pt{b}_{qc}")
                      pt3 = pt.rearrange("p (h k q) -> p h k q", h=NH, k=NKT)
                      # scores: each st tile holds both heads for one ktile;
                      # consecutive matmuls alternate PE row-groups (0-63 vs
                      # 64-127) so the array runs them concurrently
                      for kti in range(NKT):
                          k0 = b * S + kti * 128
                          st = stp.tile([128, 1024], f32, tag="st")
                          for h in range(NH):
                              hp = h * DH
                              nc.tensor.matmul(
                                  st[:, h * 512:(h + 1) * 512],
                                  kt[hp:hp + DH, k0:k0 + 128],
                                  qt[hp:hp + DH, q0:q0 + 512],
                                  start=True, stop=True,
                              )
                          nc.scalar.activation(
                              pt3[:, :, kti, :], st[:, :],
                              mybir.ActivationFunctionType.Exp,
                              scale=SCALE,
                          )
                      for h in range(NH):
                          hp = h * DH
                          av = avp.tile([128, 512], f32, tag="av")
                          for k in range(NKT):
                              nc.tensor.matmul(
                                  av[0:VSLOT, :], v4[:, h, k, :],
                                  pt3[:, h, k, :],
                                  start=(k == 0), stop=(k == NKT - 1),
                              )
                          oraw = orp.tile([VSLOT, 512], f32, tag="oraw")
                          nc.vector.tensor_copy(oraw[:, :], av[0:VSLOT, :])
                          negl = rcp.tile([1, 512], f32, tag="negl")
                          nc.scalar.activation(
                              negl[:, :], oraw[DH:VSLOT, :],
                              mybir.ActivationFunctionType.Ln)
                          recip = rcp.tile([1, 512], f32, tag="recip")
                          nc.scalar.activation(
                              recip[:, :], negl[:, :],
                              mybir.ActivationFunctionType.Exp, scale=-1.0)
                          rb = orp.tile([DH, 512], f32, tag="rb")
                          nc.gpsimd.partition_broadcast(rb[:, :], recip[:, :])
                          nc.vector.tensor_tensor(
                              onorm[hp:hp + DH, :], oraw[0:DH, :],
                              rb[:, :], op=mybir.AluOpType.mult,
                          )
                      if pending is not None:
                          emit_outproj(*pending)
                      pending = (q0, onorm)
              emit_outproj(*pending)

    nc.compile()
    return nc


def _prep_inputs(x, Wq, bq, Wk, bk, Wv, bv, Wo, bo):
    bf16 = ml_dtypes.bfloat16
    xT = np.ascontiguousarray(
        np.asarray(x, dtype=np.float32).reshape(T, D).T).astype(bf16)
    in_maps = []
    for c in range(NCORES):
        cs = slice(c * 128, (c + 1) * 128)
        in_maps.append({
            "xT": xT,
            "wq": np.ascontiguousarray(Wq[:, cs]).astype(bf16),
            "wk": np.ascontiguousarray(Wk[:, cs]).astype(bf16),
            "wv": np.ascontiguousarray(Wv[:, cs]).astype(bf16),
            "wo": np.ascontiguousarray(Wo[cs, :]).astype(bf16),
            "bq": np.ascontiguousarray(bq[cs]).reshape(128, 1).astype(np.float32),
            "bk": np.ascontiguousarray(bk[cs]).reshape(128, 1).astype(np.float32),
        })
    return in_maps


def kernel(x, Wq, bq, Wk, bk, Wv, bv, Wo, bo, _trace=False, _results=None):
    from concourse.bass_utils import run_bass_kernel_spmd

    x = np.asarray(x); Wq = np.asarray(Wq); Wk = np.asarray(Wk)
    Wv = np.asarray(Wv); Wo = np.asarray(Wo)
    bq = np.asarray(bq); bk = np.asarray(bk); bv = np.asarray(bv)
    bo = np.asarray(bo)

    if "nc" not in _CACHE:
        _CACHE["nc"] = _build_nc()
    nc = _CACHE["nc"]

    in_maps = _prep_inputs(x, Wq, bq, Wk, bk, Wv, bv, Wo, bo)
    res = run_bass_kernel_spmd(
        nc, in_maps, core_ids=list(range(NCORES)), trace=_trace)
    if _results is not None:
        _results.append(res)

    acc = np.zeros((T, D), dtype=np.float32)
    for c in range(NCORES):
        acc += np.asarray(res.results[c]["outp"], dtype=np.float32)
    acc += bv.astype(np.float32) @ Wo.astype(np.float32) + bo.astype(np.float32)
    return acc.reshape(B, S, D)



# revision 2
# speedup vs baseline: 1.1904x; 1.1904x over previous
"""MHA kernel for Trainium2, 8-core tensor-parallel (2 heads per core).

Problem (hardcoded): x [2, 2048, 1024] fp32, Wq/Wk/Wv/Wo [1024, 1024],
bq/bk/bv/bo [1024], H=16 heads, DH=64.  out = MHA(x).

Sharding: heads are split 8 ways (2 heads = 128 proj columns per core).
Each core computes its heads' attention output and a partial output
projection (row-parallel Wo); the host sums the 8 partials and adds the
closed-form bias terms (bv @ Wo + bo).

Structure: the kernel is ScalarE-bound (softmax exp = 16.8M elems/core
at 1 elem/cycle ~= 147us).  Everything else is emitted as a software
pipeline around the exp stream so the PE never idles in lumps (HAM
throttle) and the QKV/V projections ride in the PE's spare cycles
instead of a serial prologue:

  per global k-tile step s (combo c = s//16, k = s%16):
    - background proj work (kt/qt chunks, V token-tiles) per a static
      schedule with just-in-time deadlines
    - scores pair for (c, k): two K=64 matmuls on PE row-groups 0-1 /
      2-3 (run concurrently), exp on ScalarE -> pt (bf16)
    - AV accumulation steps lagged LAG behind scores (h1 one more);
      ones-column in V gives the softmax denominator in PSUM row 64
    - at combo boundaries: oraw copy, one [1,1024] Ln + Exp for both
      heads' reciprocals, gpsimd broadcast, normalize, then the
      output projection spread 2 MMs/step with bf16 staging + DMA out.
"""

import numpy as np
import ml_dtypes

D = 1024
T = 4096          # B*S tokens
S = 2048
B = 2
NH = 2            # heads per core
DH = 64
NCORES = 8
SCALE = 0.125     # 1/sqrt(DH)
NKT = S // 128    # 16 key tiles per batch
NQC = S // 512    # 4 query chunks per batch
NCK = T // 512    # 8 x^T column chunks
VSLOT = DH + 1    # 65: V columns + ones column
NCOMBO = B * NQC  # 8
LAG = 5           # AV trails scores by LAG k-tile steps

_CACHE = {}


def _build_nc(reps=1):
    import concourse.bacc as bacc
    import concourse.mybir as mybir
    import concourse.tile as tile
    from concourse.hw_specs import get_activation_tables as _gat

    # Pin Exp and Ln to the one table set that holds both, so the
    # table-load placement pass emits a single ACT_TABLE_LOAD instead of
    # thrashing between exp_and_others and natural_log every combo.
    def _pinned_tables(arch):
        out = {}
        for k, fns in _gat(arch).items():
            if k != "natural_log_exp_and_others":
                fns = {f for f in fns if f.name not in ("Exp", "Ln")}
            out[k] = fns
        return out
    bacc.get_activation_tables = _pinned_tables

    dt = mybir.dt
    f32, bf16 = dt.float32, dt.bfloat16

    nc = bacc.Bacc("TRN2", target_bir_lowering=False, debug=False,
                   num_devices=NCORES)

    xT = nc.dram_tensor("xT", [D, T], bf16, kind="ExternalInput")
    wq_d = nc.dram_tensor("wq", [D, 128], bf16, kind="ExternalInput")
    wk_d = nc.dram_tensor("wk", [D, 128], bf16, kind="ExternalInput")
    wv_d = nc.dram_tensor("wv", [D, 128], bf16, kind="ExternalInput")
    wo_d = nc.dram_tensor("wo", [128, D], bf16, kind="ExternalInput")
    bq_d = nc.dram_tensor("bq", [128, 1], f32, kind="ExternalInput")
    bk_d = nc.dram_tensor("bk", [128, 1], f32, kind="ExternalInput")
    outp = nc.dram_tensor("outp", [T, D], bf16, kind="ExternalOutput")

    with tile.TileContext(nc) as tc:
      for _rep in range(reps):
        with (
            tc.tile_pool(name="persist", bufs=1) as pp,
            tc.tile_pool(name="pt", bufs=2) as ptp,
            tc.tile_pool(name="onorm", bufs=2) as onp,
            tc.tile_pool(name="oraw", bufs=2) as orp,
            tc.tile_pool(name="recip", bufs=2) as rcp,
            tc.tile_pool(name="outsb", bufs=3) as osp,
            tc.tile_pool(name="st_ps", bufs=2, space="PSUM") as stp,
            tc.tile_pool(name="av_ps", bufs=2, space="PSUM") as avp,
            tc.tile_pool(name="op_ps", bufs=2, space="PSUM") as opp,
        ):
            # ---- weights / biases ----
            wq = pp.tile([128, D], bf16, tag="wq")
            wk = pp.tile([128, D], bf16, tag="wk")
            wv = pp.tile([128, D], bf16, tag="wv")
            wo = pp.tile([128, D], bf16, tag="wo")
            for w_sb, w_dr in ((wq, wq_d), (wk, wk_d), (wv, wv_d)):
                nc.sync.dma_start(
                    out=w_sb.rearrange("p (t c) -> p t c", c=128),
                    in_=w_dr.ap().rearrange("(t p) c -> p t c", p=128),
                )
            nc.sync.dma_start(out=wo[:, :], in_=wo_d.ap()[:, :])
            bq = pp.tile([128, 1], f32, tag="bq")
            bk = pp.tile([128, 1], f32, tag="bk")
            nc.sync.dma_start(out=bq[:, :], in_=bq_d.ap()[:, :])
            nc.sync.dma_start(out=bk[:, :], in_=bk_d.ap()[:, :])

            # ---- x^T d-tiles, batch-0 chunks first so combo 0 can
            # start as soon as chunk 0 of all 8 d-tiles lands
            xt = [pp.tile([128, T], bf16, tag=f"xt{d}", name=f"xt{d}")
                  for d in range(8)]
            for nck in range(NCK):
                cs = slice(nck * 512, (nck + 1) * 512)
                for d in range(8):
                    nc.sync.dma_start(
                        out=xt[d][:, cs],
                        in_=xT.ap()[d * 128:(d + 1) * 128, cs])

            qt = pp.tile([128, T], bf16, tag="qt")
            kt = pp.tile([128, T], bf16, tag="kt")
            wq3 = wq.rearrange("p (t c) -> p t c", c=128)
            wk3 = wk.rearrange("p (t c) -> p t c", c=128)
            wv3 = wv.rearrange("p (t c) -> p t c", c=128)

            vtm = []
            for b in range(B):
                v_sb = pp.tile([128, NH * NKT * VSLOT], bf16, tag=f"v{b}")
                v4 = v_sb.rearrange("p (h k c) -> p h k c", h=NH, k=NKT)
                nc.vector.memset(v4[:, :, :, DH:DH + 1], 1.0)
                vtm.append(v_sb)

            # ---- background-work emitters (projections) ----
            def emit_proj_chunk(proj_sb, w3, b_sb, nck):
                # one 512-col chunk of Q^T or K^T: contract D in 8 steps
                cs = slice(nck * 512, (nck + 1) * 512)
                ps = opp.tile([128, 512], f32, tag="op")
                for d in range(8):
                    nc.tensor.matmul(
                        ps[:, :], w3[:, d, :], xt[d][:, cs],
                        start=(d == 0), stop=(d == 7),
                    )
                nc.vector.tensor_scalar_add(proj_sb[:, cs], ps[:, :],
                                            b_sb[:, :])

            def emit_v_tile(b, kti):
                # one token-major V tile [128 tok, 2x64] for batch b
                v4 = vtm[b].rearrange("p (h k c) -> p h k c", h=NH, k=NKT)
                tok0 = b * S + kti * 128
                ps = opp.tile([128, 128], f32, tag="op")
                for d in range(8):
                    nc.tensor.matmul(
                        ps[:, :], xt[d][:, tok0:tok0 + 128], wv3[:, d, :],
                        start=(d == 0), stop=(d == 7),
                    )
                nc.vector.tensor_copy(
                    v4[:, :, kti, 0:DH],
                    ps.rearrange("p (h c) -> p h c", h=NH)[:, :, :],
                )

            def bg_item(kind, a, bb):
                if kind == "kt":
                    emit_proj_chunk(kt, wk3, bk, a)
                elif kind == "qt":
                    emit_proj_chunk(qt, wq3, bq, a)
                else:
                    emit_v_tile(bb, a)

            # static schedule: step -> list of background items.
            # deadlines: scores(c,k) needs kt chunk (b*4 + k//4) and the
            # qt chunk for (b,qc) at step 16c+k; av(c,k) at step
            # 16c+k+LAG needs v(b,k).
            bg = {}
            def at(step, kind, a, bb=0):
                bg.setdefault(step, []).append((kind, a, bb))
            # combo 0: remaining b0 kt chunks + all b0 v tiles
            at(1, "kt", 1); at(4, "kt", 2); at(7, "kt", 3)
            for kti in range(NKT):
                at(kti, "v", kti, 0)
            at(13, "qt", 1)                      # qc1 due step 16
            at(24, "qt", 2); at(45, "qt", 3)     # qc2/qc3 due 32/48
            for j in range(4):                   # kt b1 due step 64+4j
                at(18 + 3 * j, "kt", 4 + j)
            for kti in range(4):                 # v b1 k0-3 due ~69
                at(34 + 3 * kti, "v", kti, 1)
            for kti in range(4, 12):             # due 68+k
                at(48 + 2 * (kti - 4), "v", kti, 1)
            for kti in range(12, 16):            # due 80+k
                at(65 + 2 * (kti - 12), "v", kti, 1)
            at(47, "qt", 4)                      # b1 qc0 due 64
            at(72, "qt", 5); at(86, "qt", 6); at(102, "qt", 7)

            # prologue: minimum to start combo 0
            emit_proj_chunk(kt, wk3, bk, 0)
            emit_proj_chunk(qt, wq3, bq, 0)

            # ---- attention pipeline state ----
            combos = [(b, qc) for b in range(B) for qc in range(NQC)]
            pt_tiles = [None] * NCOMBO          # [128, NH*NKT*512] bf16
            av_tiles = [[None, None] for _ in range(NCOMBO)]
            oraw_tiles = [None] * NCOMBO        # [VSLOT, 1024] f32 SBUF
            onorm_tiles = [None] * NCOMBO

            def emit_scores_exp(c, k):
                b, qc = combos[c]
                q0 = b * S + qc * 512
                k0 = b * S + k * 128
                if k == 0:
                    pt_tiles[c] = ptp.tile([128, NH * NKT * 512], bf16,
                                           tag="pt", name=f"pt{c}")
                pt3 = pt_tiles[c].rearrange("p (h k q) -> p h k q",
                                            h=NH, k=NKT)
                st = stp.tile([128, 1024], f32, tag="st")
                for h in range(NH):
                    hp = h * DH
                    nc.tensor.matmul(
                        st[:, h * 512:(h + 1) * 512],
                        kt[hp:hp + DH, k0:k0 + 128],
                        qt[hp:hp + DH, q0:q0 + 512],
                        start=True, stop=True,
                    )
                nc.scalar.activation(
                    pt3[:, :, k, :], st[:, :],
                    mybir.ActivationFunctionType.Exp,
                    scale=SCALE,
                )

            def emit_av_step(h, g):
                # g-th global AV k-step for head h (g = 16*c + k)
                if not (0 <= g < 16 * NCOMBO):
                    return
                c, k = divmod(g, 16)
                b, qc = combos[c]
                v4 = vtm[b].rearrange("p (h k c) -> p h k c", h=NH, k=NKT)
                pt3 = pt_tiles[c].rearrange("p (h k q) -> p h k q",
                                            h=NH, k=NKT)
                if k == 0:
                    av_tiles[c][h] = avp.tile([128, 512], f32, tag="av",
                                              name=f"av{c}_{h}")
                av = av_tiles[c][h]
                nc.tensor.matmul(
                    av[0:VSLOT, :], v4[:, h, k, :], pt3[:, h, k, :],
                    start=(k == 0), stop=(k == NKT - 1),
                )
                if k == NKT - 1:
                    # stage this head's O_raw^T + denom row into the
                    # shared [65, 1024] SBUF tile (frees the PSUM bank)
                    if oraw_tiles[c] is None:
                        oraw_tiles[c] = orp.tile([VSLOT, 1024], f32,
                                                 tag="oraw",
                                                 name=f"oraw{c}")
                    orw = oraw_tiles[c]
                    nc.vector.tensor_copy(
                        orw[:, h * 512:(h + 1) * 512], av[0:VSLOT, :])

            def emit_norm(c):
                # both heads' denominators sit in oraw row 64, cols
                # 0-511 (h0) and 512-1023 (h1): one Ln + one Exp
                orw = oraw_tiles[c]
                negl = rcp.tile([1, 1024], f32, tag="negl")
                nc.scalar.activation(
                    negl[:, :], orw[DH:VSLOT, :],
                    mybir.ActivationFunctionType.Ln)
                recip = rcp.tile([1, 1024], f32, tag="recip")
                nc.scalar.activation(
                    recip[:, :], negl[:, :],
                    mybir.ActivationFunctionType.Exp, scale=-1.0)
                rb = rcp.tile([DH, 1024], f32, tag="rb")
                nc.gpsimd.partition_broadcast(rb[:, :], recip[:, :])
                onorm_tiles[c] = onp.tile([128, 512], bf16, tag="onorm",
                                          name=f"onorm{c}")
                onorm = onorm_tiles[c]
                for h in range(NH):
                    hp = h * DH
                    nc.vector.tensor_tensor(
                        onorm[hp:hp + DH, :],
                        orw[0:DH, h * 512:(h + 1) * 512],
                        rb[:, h * 512:(h + 1) * 512],
                        op=mybir.AluOpType.mult,
                    )

            def emit_outproj_pair(c, i):
                # i-th pair (of 4) of output-projection matmuls for c
                b, qc = combos[c]
                q0 = b * S + qc * 512
                onorm = onorm_tiles[c]
                for j in range(2):
                    idx = 2 * i + j
                    s4, jc = divmod(idx, 2)
                    op = opp.tile([128, 512], f32, tag="op",
                                  name=f"op{c}_{idx}")
                    nc.tensor.matmul(
                        op[:, :], onorm[:, s4 * 128:(s4 + 1) * 128],
                        wo[:, jc * 512:(jc + 1) * 512],
                        start=True, stop=True,
                    )
                    osb = osp.tile([128, 512], bf16, tag="outsb",
                                   name=f"osb{c}_{idx}")
                    nc.vector.tensor_copy(osb[:, :], op[:, :])
                    r0 = q0 + s4 * 128
                    nc.sync.dma_start(
                        out=outp.ap()[r0:r0 + 128,
                                      jc * 512:(jc + 1) * 512],
                        in_=osb[:, :],
                    )

            # ---- main pipeline ----
            NSTEP = 16 * NCOMBO
            for step in range(NSTEP + LAG + 12):
                for kind, a, bb in bg.get(step, ()):
                    bg_item(kind, a, bb)
                if step < NSTEP:
                    c, k = divmod(step, 16)
                    emit_scores_exp(c, k)
                emit_av_step(0, step - LAG)
                emit_av_step(1, step - LAG - 1)
                c2, k2 = divmod(step, 16)
                if 1 <= c2 <= NCOMBO and k2 == 7:
                    emit_norm(c2 - 1)           # prev combo's recip
                if 1 <= c2 <= NCOMBO and 10 <= k2 <= 13:
                    emit_outproj_pair(c2 - 1, k2 - 10)

    nc.compile()
    return nc


def _prep_inputs(x, Wq, bq, Wk, bk, Wv, bv, Wo, bo):
    bf16 = ml_dtypes.bfloat16
    xT = np.ascontiguousarray(
        np.asarray(x, dtype=np.float32).reshape(T, D).T).astype(bf16)
    in_maps = []
    for c in range(NCORES):
        cs = slice(c * 128, (c + 1) * 128)
        in_maps.append({
            "xT": xT,
            "wq": np.ascontiguousarray(Wq[:, cs]).astype(bf16),
            "wk": np.ascontiguousarray(Wk[:, cs]).astype(bf16),
            "wv": np.ascontiguousarray(Wv[:, cs]).astype(bf16),
            "wo": np.ascontiguousarray(Wo[cs, :]).astype(bf16),
            "bq": np.ascontiguousarray(bq[cs]).reshape(128, 1).astype(np.float32),
            "bk": np.ascontiguousarray(bk[cs]).reshape(128, 1).astype(np.float32),
        })
    return in_maps


def kernel(x, Wq, bq, Wk, bk, Wv, bv, Wo, bo, _trace=False, _results=None):
    from concourse.bass_utils import run_bass_kernel_spmd

    x = np.asarray(x); Wq = np.asarray(Wq); Wk = np.asarray(Wk)
    Wv = np.asarray(Wv); Wo = np.asarray(Wo)
    bq = np.asarray(bq); bk = np.asarray(bk); bv = np.asarray(bv)
    bo = np.asarray(bo)

    if "nc" not in _CACHE:
        _CACHE["nc"] = _build_nc()
    nc = _CACHE["nc"]

    in_maps = _prep_inputs(x, Wq, bq, Wk, bk, Wv, bv, Wo, bo)
    res = run_bass_kernel_spmd(
        nc, in_maps, core_ids=list(range(NCORES)), trace=_trace)
    if _results is not None:
        _results.append(res)

    acc = np.zeros((T, D), dtype=np.float32)
    for c in range(NCORES):
        acc += np.asarray(res.results[c]["outp"], dtype=np.float32)
    acc += bv.astype(np.float32) @ Wo.astype(np.float32) + bo.astype(np.float32)
    return acc.reshape(B, S, D)
